# revision 34
# baseline (speedup 1.0000x reference)
"""Trainium2 Bass kernel for CTRAttention (spatial graph + alpha*softmax attention).

Math (per frame t of B*T frames, J=17 joints, 8 heads x 32 dims):
  qkv = x @ w_qkv.T                      [tok, 768]
  per (frame, head): s = q @ k.T * SCALE; A = softmax(s)
  attn = shared + alpha * A              [17, 17]
  out  = attn @ v                        [17, 256]
  y    = out @ w_proj.T + b_proj         [tok, 512]

Sharding: data-parallel over B across 8 cores (2 batches each).

Per-core layout strategy (all matmuls bf16, fp32 PSUM accumulation):
  - host passes xT = x.T [512, NTOK] bf16; outputs yT [512, NTOK] fp32
    (host transposes back - free, not on device).
  - stage 1: qkT[(h,c), tok] (4 tiles of 128 partitions) and v tiles
    [119 tokens, 256] aligned to 7-frame groups.
  - stage 2 scores: per 4-frame group and head, block-diagonal operands
    built via SBUF->SBUF DMAs (DMA APs may step partitions+free jointly;
    compute engines may not):
      kstack_h [ (t4,c32)=128, k32 ]   (k-slab stacked by frame)
      qBD_h    [ (t4,c32)=128, (t4,j17)=68 ]  block-diagonal
    MM out = scoresT [ (h4,k32), (t,j) ] with tile_position column tiling.
    softmax over k (partitions): exp on ACT, per-head sums via a constant
    ones-mask matmul (M=128 -> full replicated layout, includes 1/alpha),
    DVE reciprocal, one broadcast multiply, plus sharedT add -> attnT.
  - stage 2 A@V: per 7-frame group and head, attnBD7 [119,119] block diag
    (DMA-built), lhsT = direct v slice [119, 32] -> out_avT [(h,c), tok].
  - stage 3: yT[d2, tok] = w_projT-stationary matmuls, bias per-partition
    during PSUM eviction.
"""

import math

import numpy as np

B, T, J = 16, 243, 17
DIM = 512
NUM_HEADS = 8
INTER = 256
HEAD_DIM = 32
SCALE = HEAD_DIM ** (-0.5)
N_CORES = 8
B_LOC = B // N_CORES           # 2
F = B_LOC * T                  # 486 frames per core
NTOK = F * J                   # 8262 tokens per core

SB_FRAMES = 56                 # superblock: 14 groups of 4 = 8 groups of 7
SB_TOK = SB_FRAMES * J         # 952


def _group_sizes(total, g):
    out = []
    n = total
    while n > 0:
        out.append(min(g, n))
        n -= g
    return out


def build_nc(n_frames=F, sim_safe=False):
    """Build the per-core Bass program.

    sim_safe=True adds a serializing dependency chain over the custom
    (partition-crossing) build DMAs and each group's last consumer so
    CoreSim's conservative shadow-memory checker can resolve read versions.
    The rust checker over-approximates partition-crossing linear APs as wide
    byte intervals and reports false conflicts otherwise. Data flow is
    identical; use sim_safe only for simulator validation, not hardware.
    """
    import concourse.bass as bass
    import concourse.bacc as bacc
    import concourse.mybir as mybir
    from concourse import tile

    bf16 = mybir.dt.bfloat16
    f32 = mybir.dt.float32
    AF = mybir.ActivationFunctionType
    ALU = mybir.AluOpType

    NTOK = n_frames * J

    # detect_race_conditions only affects CoreSim; the rust checker models
    # partition-crossing linear DMA APs (our block-diagonal builds) as
    # overlapping byte ranges and reports false positives.
    # Bacc (not raw Bass): its compile() pass splits multi-wait DMAs, which
    # the HWDGE descriptor format (1 wait slot) requires.
    nc = bacc.Bacc("TRN2", debug=False, detect_race_conditions=False)

    # ---- DRAM I/O ----
    xT = nc.dram_tensor("xT", [DIM, NTOK], bf16, kind="ExternalInput")
    wqk = nc.dram_tensor("wqk", [DIM, DIM], bf16, kind="ExternalInput")     # w_qkv[:512].T
    wv = nc.dram_tensor("wv", [DIM, INTER], bf16, kind="ExternalInput")     # w_qkv[512:].T
    wp = nc.dram_tensor("wp", [INTER, DIM], bf16, kind="ExternalInput")     # w_proj.T
    bias = nc.dram_tensor("bias", [128, 4], f32, kind="ExternalInput")      # b_proj[128m+p]
    sharedT = nc.dram_tensor("sharedT", [128, J], bf16, kind="ExternalInput")
    onesmask = nc.dram_tensor("onesmask", [128, 128], bf16, kind="ExternalInput")
    yT = nc.dram_tensor("yT", [DIM, NTOK], f32, kind="ExternalOutput")

    # superblock split of the core's tokens
    sb_frames = _group_sizes(n_frames, SB_FRAMES)   # [56]*8 + [38]
    n_sb = len(sb_frames)

    with tile.TileContext(nc) as tc:
        with (
            tc.tile_pool(name="const", bufs=1) as cpool,
            tc.tile_pool(name="big", bufs=1) as bpool,
            tc.tile_pool(name="ps1", bufs=1, space="PSUM") as ps1,
            tc.tile_pool(name="ps2", bufs=1, space="PSUM") as ps2,
        ):
            # ---- constants in SBUF ----
            wqk_t = [[cpool.tile([128, 128], bf16, tag=f"wqk_{c}_{m}", name=f"wqk_{c}_{m}")
                      for m in range(4)] for c in range(4)]
            for c in range(4):
                for m in range(4):
                    nc.sync.dma_start(wqk_t[c][m][:, :],
                                      wqk.ap()[128 * c:128 * (c + 1), 128 * m:128 * (m + 1)])
            wv_t = [cpool.tile([128, INTER], bf16, tag=f"wv_{c}", name=f"wv_{c}") for c in range(4)]
            for c in range(4):
                nc.sync.dma_start(wv_t[c][:, :], wv.ap()[128 * c:128 * (c + 1), :])
            wp_t = [[cpool.tile([128, 128], bf16, tag=f"wp_{i}_{m}", name=f"wp_{i}_{m}")
                     for m in range(4)] for i in range(2)]
            for i in range(2):
                for m in range(4):
                    nc.sync.dma_start(wp_t[i][m][:, :], wp.ap()[128 * i:128 * (i + 1), 128 * m:128 * (m + 1)])
            bias_t = cpool.tile([128, 4], f32, tag="bias", name="bias_t")
            nc.sync.dma_start(bias_t[:, :], bias.ap())
            sharedT_t = cpool.tile([128, J], bf16, tag="sharedT", name="sharedT_t")
            nc.sync.dma_start(sharedT_t[:, :], sharedT.ap())
            ones_t = cpool.tile([128, 128], bf16, tag="onesmask", name="ones_t")
            nc.sync.dma_start(ones_t[:, :], onesmask.ap())

            # ---- ring buffers (explicit 2-deep rings across superblocks) ----
            R = 2
            xT_r = [[bpool.tile([128, SB_TOK], bf16, tag=f"xT_{r}_{c}", name=f"xT_{r}_{c}")
                     for c in range(4)] for r in range(R)]
            qkT_r = [[bpool.tile([128, SB_TOK], bf16, tag=f"qkT_{r}_{m}", name=f"qkT_{r}_{m}")
                      for m in range(4)] for r in range(R)]
            v_r = [bpool.tile([128, 8 * INTER], bf16, tag=f"v_{r}", name=f"v_{r}") for r in range(R)]
            attnT_r = [[bpool.tile([128, SB_TOK], bf16, tag=f"attnT_{r}_{g}", name=f"attnT_{r}_{g}")
                        for g in range(2)] for r in range(R)]
            avT_r = [[bpool.tile([128, SB_TOK], bf16, tag=f"avT_{r}_{g}", name=f"avT_{r}_{g}")
                      for g in range(2)] for r in range(R)]

            # small stage-2 rings (deeper, cycled per group)
            R2 = 3
            kstack_r = [[bpool.tile([128, 32], bf16, tag=f"kst_{r}_{h}", name=f"kst_{r}_{h}")
                         for h in range(8)] for r in range(R2)]
            qBD_r = [[bpool.tile([128, 68], bf16, tag=f"qBD_{r}_{h}", name=f"qBD_{r}_{h}")
                      for h in range(8)] for r in range(R2)]
            aBD_r = [[bpool.tile([128, 7 * J], bf16, tag=f"aBD_{r}_{h}", name=f"aBD_{r}_{h}")
                      for h in range(8)] for r in range(R2)]
            exp_r = [bpool.tile([128, 136], bf16, tag=f"exp_{r}", name=f"exp_{r}") for r in range(R2)]
            rec_r = [bpool.tile([128, 136], bf16, tag=f"rec_{r}", name=f"rec_{r}") for r in range(R2)]
            t1_r = [bpool.tile([128, 136], bf16, tag=f"t1_{r}", name=f"t1_{r}") for r in range(R2)]
            yev_r = [bpool.tile([128, 512], f32, tag=f"yev_{r}", name=f"yev_{r}") for r in range(4)]

            # zero the block-diagonal rings once (diagonals are rewritten
            # every iteration; off-diagonal zeros persist)
            for r in range(R2):
                for h in range(8):
                    nc.vector.memset(kstack_r[r][h][:, :], 0.0)
                    nc.vector.memset(qBD_r[r][h][:, :], 0.0)
                    nc.vector.memset(aBD_r[r][h][:, :], 0.0)

            # PSUM tiles (distinct tags -> distinct banks; full-bank sized so
            # matmul outputs never cross a bank boundary)
            psA = [ps1.tile([128, 512], f32, tag=f"psA{i}", name=f"psA{i}") for i in range(3)]  # qk/v/proj
            psS = [ps2.tile([128, 512], f32, tag=f"psS{i}", name=f"psS{i}") for i in range(2)]  # scores+sums
            psV = [ps2.tile([128, 512], f32, tag=f"psV{i}", name=f"psV{i}") for i in range(2)]  # avT

            def evict(dst_ap, src_ap, eng):
                # dedicated engine per destination tensor class: keeps the
                # number of distinct producer semaphores per consumer DMA low
                # (walrus: "Too many sync wait commands" otherwise)
                if eng == "v":
                    return nc.vector.tensor_copy(dst_ap, src_ap)
                return nc.scalar.copy(dst_ap, src_ap)



            psA_i = [0]

            def next_psA():
                t = psA[psA_i[0] % 3]
                psA_i[0] += 1
                return t

            tok0 = 0
            g4_idx = 0
            g7_idx = 0
            for sb in range(n_sb):
                fsb = sb_frames[sb]
                wtok = fsb * J
                r = sb % R
                xTs = xT_r[r]
                qkTs = qkT_r[r]
                vs = v_r[r]
                attnTs = attnT_r[r]
                avTs = avT_r[r]

                # ---------- stage 1: load xT window ----------
                for c in range(4):
                    nc.sync.dma_start(xTs[c][:, 0:wtok],
                                      xT.ap()[128 * c:128 * (c + 1), tok0:tok0 + wtok])

                # q/k projection: 4 d-tiles, accumulate 4 c-chunks
                nhalf = (wtok + 511) // 512
                for m in range(4):
                    for nh in range(nhalf):
                        n0 = (wtok * nh) // nhalf
                        n1 = (wtok * (nh + 1)) // nhalf
                        ps = next_psA()
                        for c in range(4):
                            nc.tensor.matmul(ps[:, 0:n1 - n0], wqk_t[c][m][:, :],
                                             xTs[c][:, n0:n1],
                                             start=(c == 0), stop=(c == 3))
                        evict(qkTs[m][:, n0:n1], ps[:, 0:n1 - n0], "v")

                # v projection, frame-aligned 7-frame groups
                g7s = _group_sizes(fsb, 7)
                gt = 0
                for gi, g in enumerate(g7s):
                    gw = g * J
                    ps = next_psA()
                    for c in range(4):
                        nc.tensor.matmul(ps[0:gw, 0:INTER], xTs[c][:, gt:gt + gw],
                                         wv_t[c][:, :], start=(c == 0), stop=(c == 3))
                    evict(vs[0:gw, INTER * gi:INTER * (gi + 1)], ps[0:gw, 0:INTER], "s")
                    gt += gw

                # ---------- stage 2a: scores + softmax per 4-frame group ----------
                g4s = _group_sizes(fsb, 4)
                ft = 0   # frame offset within superblock
                for gi, g in enumerate(g4s):
                    rr = g4_idx % R2
                    gw = g * J
                    base = ft * J          # token offset within superblock
                    ksts = kstack_r[rr]
                    qBDs = qBD_r[rr]
                    expt = exp_r[rr]
                    rect = rec_r[rr]
                    t1t = t1_r[rr]
                    psc = psS[g4_idx % 2]

                    for hg in range(2):
                        for h2 in range(4):
                            h = 4 * hg + h2
                            # kstack_h[32t+c, k] = qkT[2+hg][32h2+c, base+17t+k]
                            # tile [128,32]: element (t,c,k) at linear (32t+c)*32 + k
                            # hardware DMA APs: dim0 = whole partitions only,
                            # dims>=1 = bytes within partition. Diagonal
                            # placement therefore needs one plain DMA per
                            # frame block.
                            kst = ksts[h]
                            qb = qBDs[h]
                            for t in range(g):
                                nc.sync.dma_start(
                                    kst[32 * t:32 * t + 32, 0:J],
                                    qkTs[2 + hg][32 * h2:32 * h2 + 32,
                                                 base + J * t:base + J * (t + 1)])
                                nc.sync.dma_start(
                                    qb[32 * t:32 * t + 32, J * t:J * (t + 1)],
                                    qkTs[hg][32 * h2:32 * h2 + 32,
                                             base + J * t:base + J * (t + 1)])

                    # scores MMs: out[32h2+k, (t,j)] in psc[:, 68*hg : 68*hg+gw]
                    for hg in range(2):
                        for h2 in range(4):
                            h = 4 * hg + h2
                            nc.tensor.matmul(
                                psc[32 * h2:32 * (h2 + 1), 68 * hg:68 * hg + gw],
                                ksts[h][:, :], qBDs[h][:, 0:gw],
                                start=True, stop=True,
                                tile_position=(0, 32 * h2))
                    # exp (scale folded)
                    for hg in range(2):
                        nc.scalar.activation(expt[:, 68 * hg:68 * hg + gw],
                                             psc[:, 68 * hg:68 * hg + gw],
                                             AF.Exp, scale=float(SCALE))
                    # per-head sums (replicated over all 128 rows), 1/alpha folded
                    for hg in range(2):
                        nc.tensor.matmul(psc[:, 136 + 68 * hg:136 + 68 * hg + gw],
                                         ones_t[:, :], expt[:, 68 * hg:68 * hg + gw],
                                         start=True, stop=True)
                    with nc.allow_low_precision(reason="softmax denominators tolerate bf16"):
                        for hg in range(2):
                            nc.vector.reciprocal(rect[:, 68 * hg:68 * hg + gw],
                                                 psc[:, 136 + 68 * hg:136 + 68 * hg + gw])
                    if gw == 68:
                        nc.vector.tensor_tensor(t1t[:, 0:136], expt[:, 0:136],
                                                rect[:, 0:136], op=ALU.mult)
                    else:
                        for hg in range(2):
                            nc.vector.tensor_tensor(t1t[:, 68 * hg:68 * hg + gw],
                                                    expt[:, 68 * hg:68 * hg + gw],
                                                    rect[:, 68 * hg:68 * hg + gw],
                                                    op=ALU.mult)
                    # add sharedT (broadcast over t) -> attnT
                    for hg in range(2):
                        sh_b = sharedT_t[:, :].unsqueeze(1).broadcast_to([128, g, J])
                        a_in = t1t[:, 68 * hg:68 * hg + gw].rearrange("p (t j) -> p t j", t=g)
                        a_out = attnTs[hg][:, base:base + gw].rearrange("p (t j) -> p t j", t=g)
                        nc.vector.tensor_tensor(a_out, a_in, sh_b, op=ALU.add)
                    ft += g
                    g4_idx += 1

                # ---------- stage 2b: A@V per 7-frame group ----------
                gt = 0
                for gi, g in enumerate(g7s):
                    rr = g7_idx % R2
                    gw = g * J
                    aBDs = aBD_r[rr]
                    psv = psV[g7_idx % 2]
                    for hg in range(2):
                        for h2 in range(4):
                            h = 4 * hg + h2
                            ab = aBDs[h]
                            # aBD[17t+k, 17t+j] = attnT[hg][32h2+k, gt+17t+j]
                            for t in range(g):
                                nc.sync.dma_start(
                                    ab[J * t:J * (t + 1), J * t:J * (t + 1)],
                                    attnTs[hg][32 * h2:32 * h2 + J,
                                               gt + J * t:gt + J * (t + 1)])
                    for hg in range(2):
                        for h2 in range(4):
                            h = 4 * hg + h2
                            nc.tensor.matmul(
                                psv[32 * h2:32 * (h2 + 1), 119 * hg:119 * hg + gw],
                                vs[0:gw, INTER * gi + 32 * h:INTER * gi + 32 * (h + 1)],
                                aBDs[h][0:gw, 0:gw],
                                start=True, stop=True,
                                tile_position=(0, 32 * h2))
                    for hg in range(2):
                        evict(avTs[hg][:, gt:gt + gw], psv[:, 119 * hg:119 * hg + gw],
                              "v" if hg == 0 else "s")
                    gt += gw
                    g7_idx += 1

                # ---------- stage 3: projection ----------
                for m in range(4):
                    for nh in range(nhalf):
                        n0 = (wtok * nh) // nhalf
                        n1 = (wtok * (nh + 1)) // nhalf
                        ps = next_psA()
                        for i in range(2):
                            nc.tensor.matmul(ps[:, 0:n1 - n0], wp_t[i][m][:, :],
                                             avTs[i][:, n0:n1],
                                             start=(i == 0), stop=(i == 1))
                        yev = yev_r[(m * nhalf + nh) % 4]
                        # bias add during eviction (per-partition scalar)
                        if (m + nh) % 2 == 0:
                            nc.vector.tensor_scalar_add(yev[:, 0:n1 - n0], ps[:, 0:n1 - n0],
                                                        bias_t[:, m:m + 1])
                        else:
                            nc.scalar.activation(yev[:, 0:n1 - n0], ps[:, 0:n1 - n0],
                                                 AF.Identity, bias=bias_t[:, m:m + 1])
                        nc.sync.dma_start(
                            yT.ap()[128 * m:128 * (m + 1), tok0 + n0:tok0 + n1],
                            yev[:, 0:n1 - n0])

                tok0 += wtok

    nc.compile()
    return nc


_NC_CACHE = {}


def _get_nc():
    if "nc" not in _NC_CACHE:
        _NC_CACHE["nc"] = build_nc()
    return _NC_CACHE["nc"]


def _host_prep(x, w_qkv, w_proj, b_proj, shared_attn, alpha):
    import ml_dtypes
    bf = ml_dtypes.bfloat16
    alpha = float(np.asarray(alpha))
    wqk = np.ascontiguousarray(w_qkv[:512].T.astype(bf))          # [512, 512]
    wv = np.ascontiguousarray(w_qkv[512:].T.astype(bf))           # [512, 256]
    wp = np.ascontiguousarray(w_proj.T.astype(bf))                # [256, 512]
    bias = np.ascontiguousarray(
        np.asarray(b_proj, np.float32).reshape(4, 128).T)         # [128, 4]
    sharedT = np.zeros((128, J), np.float32)
    sa = np.asarray(shared_attn, np.float32)
    for h in range(4):
        sharedT[32 * h:32 * h + J, :] = sa.T                      # [k, j] = shared[j,k]
    onesmask = np.zeros((128, 128), np.float32)
    for h in range(4):
        onesmask[32 * h:32 * h + J, 32 * h:32 * (h + 1)] = 1.0 / alpha
    ins = []
    for core in range(N_CORES):
        xl = np.asarray(x[B_LOC * core:B_LOC * (core + 1)], np.float32).reshape(NTOK, DIM)
        xTl = np.ascontiguousarray(xl.T.astype(bf))               # [512, NTOK]
        ins.append({
            "xT": xTl, "wqk": wqk, "wv": wv, "wp": wp,
            "bias": bias.astype(np.float32),
            "sharedT": sharedT.astype(bf),
            "onesmask": onesmask.astype(bf),
        })
    return ins


def kernel(x, w_qkv, w_proj, b_proj, shared_attn, alpha, _trace=False):
    from concourse import bass_utils
    nc = _get_nc()
    in_maps = _host_prep(x, w_qkv, w_proj, b_proj, shared_attn, alpha)
    res = bass_utils.run_bass_kernel_spmd(nc, in_maps, core_ids=list(range(N_CORES)),
                                          trace=_trace)
    outs = []
    for core in range(N_CORES):
        yTl = np.asarray(res.results[core]["yT"], np.float32)     # [512, NTOK]
        outs.append(yTl.T.reshape(B_LOC, T, J, DIM))
    full = np.concatenate(outs, axis=0)
    if _trace:
        return full, res
    return full
